# revision 50
# baseline (speedup 1.0000x reference)
"""Trainium2 Bass kernel for nn_LocalizedFiltering (fused cat-conv2d x2 + residual + RMSNorm).

Strategy: sequence-parallel across 8 NeuronCores (one sequence of 2048 tokens +
1 cache row per core) -- no collectives needed.

Layer 1 runs in fp8-e4m3 DoubleRow mode: every matmul carries TWO contraction
k-tiles (the DoubleRow groups are adjacent k-row pairs -- plain strided APs),
streaming 256 contraction rows in the time bf16 streams 512. Accuracy is
recovered with hi+lo e4m3 pairs on both operands (x ~ xh+xl, W ~ Wh+Wl,
power-of-two pre-scales) accumulating the three significant products
xh*Wh + xl*Wh + xh*Wl in fp32 PSUM. Layer 2 stays bf16: its causal shift
lives on the lhsT (weight-load) side, whose fp8 ISA path requires aligned
strides/offsets that a +-1 token window cannot satisfy.

Layout (no on-chip transposes):
  Phase A (layer 1), feature-major: psum[feat, tok]; act drain descales, adds
    b1, emits xt2 bf16 -- exactly the lhsT layout phase B needs.
  Phase B (layer 2), token-major bf16: psum[tok, feat]; residual + bias via
    host-folded xres = x + b2; RMSNorm on token partitions; direct DMA out.
ln_weight is applied exactly on the host (out *= ln_weight).
"""

import os

import numpy as np
import ml_dtypes

BS, L, D, CACHE = 8, 2048, 2048, 64
T = BS * L
H = D // 2          # 1024
EPS = 1e-6
NCORES = 8
BLK = 512           # token block (= one PSUM bank of fp32)
NBLK = L // BLK     # 4
KP1 = D // 256      # 8 contraction k-PAIRS, layer 1
KP2 = H // 256      # 4 contraction k-pairs, layer 2
KT2 = H // 128      # 8 contraction tiles, layer 2 (bf16 phase B)
QT1 = H // 128      # 8 output-feature tiles, layer 1 (per half)
NTT = L // 128      # 16 token tiles, layer 2
FS = 512            # feature slice, layer 2 output

# power-of-two quantization scales (inputs ~N(0,1), weights ~N(0,0.02))
SX = 32.0           # layer-1 input scale
S2 = 32.0           # layer-2 input (o1) scale
SW1 = 2048.0
SW2 = 2048.0
LP2 = L             # xt2 row length (win0 never reads col L; pow2 strides)
EPS_EFF = EPS * (S2 * SW2) ** 2   # folds the psum descale into RMSNorm

TRACE = bool(int(os.environ.get("BASS_KERNEL_TRACE", "0")))
LAST_EXEC_NS = None
LAST_RESULTS = None

_NC_CACHE = {}


def _build_bass():
    if "nc" in _NC_CACHE:
        return _NC_CACHE["nc"]

    import concourse.bacc as bacc
    import concourse.tile as tile
    import concourse.mybir as mybir

    fp32 = mybir.dt.float32
    bf16 = mybir.dt.bfloat16
    f8 = mybir.dt.float8e4
    Act = mybir.ActivationFunctionType
    DR = mybir.MatmulPerfMode.DoubleRow

    nc = bacc.Bacc("TRN2", target_bir_lowering=False)

    x1h = nc.declare_dram_parameter("x1h", [D, L + 1], f8, isOutput=False)
    x1l = nc.declare_dram_parameter("x1l", [D, L + 1], f8, isOutput=False)
    xres = nc.declare_dram_parameter("xres", [L, D], bf16, isOutput=False)
    c2h = nc.declare_dram_parameter("c2h", [H, 1], f8, isOutput=False)
    c2l = nc.declare_dram_parameter("c2l", [H, 1], f8, isOutput=False)
    w1h = nc.declare_dram_parameter("w1h", [D, D], f8, isOutput=False)
    w1l = nc.declare_dram_parameter("w1l", [D, D], f8, isOutput=False)
    w2h = nc.declare_dram_parameter("w2h", [H, 2 * D], f8, isOutput=False)
    w2l = nc.declare_dram_parameter("w2l", [H, 2 * D], f8, isOutput=False)
    b1s = nc.declare_dram_parameter("b1s", [H, 1], fp32, isOutput=False)
    out = nc.declare_dram_parameter("out", [L, D], bf16, isOutput=True)

    with tile.TileContext(nc) as tc, \
            tc.tile_pool(name="wpool", bufs=1) as wpool, \
            tc.tile_pool(name="x1p", bufs=2) as x1p, \
            tc.tile_pool(name="xt2p", bufs=1) as xt2p, \
            tc.tile_pool(name="t2p", bufs=8) as t2p, \
            tc.tile_pool(name="xresp", bufs=1) as xresp, \
            tc.tile_pool(name="rowp", bufs=2) as rowp, \
            tc.tile_pool(name="scr", bufs=1) as scr, \
            tc.tile_pool(name="tmp", bufs=2) as tmp, \
            tc.tile_pool(name="const", bufs=1) as const, \
            tc.tile_pool(name="psp", bufs=8, space="PSUM") as psp:

        epssb = const.tile([128, 1], fp32)
        nc.vector.memset(epssb, EPS_EFF)

        # startup tiles for k-pair 0 (hi parts gate the first instructions)
        wfirst = const.tile([128, 2, 384], f8, name="wfirst")
        nc.sync.dma_start(
            out=wfirst,
            in_=w1h[0:256, 0:384].rearrange("(i p) c -> p i c", p=128))
        wAh = const.tile([128, 2, H], f8, name="wAh")
        b1sb = const.tile([128, QT1, 1], fp32)
        # xt2 hi/lo fp8 with BOTH window shifts materialized: [...] win=0 col t
        # holds xt2[t], win=1 col t holds xt2[t+1]; pair/win strides stay
        # 4-aligned so phase-B fp8 LDWEIGHTS APs are legal.
        xt2wh = xt2p.tile([128, KP2, 2, 2, LP2], f8)   # [kp, win, pair, col]
        xt2wl = xt2p.tile([128, KP2, 2, 2, LP2], f8)

        # ---------------- Phase A: layer 1 -> xt2 hi/lo fp8 -----------------
        # W1 as 4+4 quad tiles [128, 4, D] (hi and lo); each DoubleRow lhsT is
        # a k-row PAIR [128, 2, 128] sliced from a quad. The same 8 slots are
        # later reused by the W2 pair tiles [128, 2, 2D].
        NQ = KP1 // 2  # 4 quads
        w1hq, w1lq = [], []
        for j in range(NQ):
            w1hq.append(wpool.tile([128, 4, D], f8, tag=f"w{j}", name=f"w1h_{j}"))
            w1lq.append(wpool.tile([128, 4, D], f8, tag=f"w{4 + j}", name=f"w1l_{j}"))

        x1kh0, x1kl0 = [], []

        def x1dma(dst, src, kp, b):
            c0 = b * BLK
            nc.sync.dma_start(
                out=dst,
                in_=src[kp * 256:(kp + 1) * 256, c0:c0 + BLK + 1].rearrange(
                    "(i p) f -> p i f", p=128))

        for kp in range(KP1):
            xh = x1p.tile([128, 2, BLK + 1], f8, tag=f"xh{kp}", name=f"x1h_0_{kp}")
            x1dma(xh, x1h, kp, 0)
            x1kh0.append(xh)
            if kp == 0:
                nc.sync.dma_start(
                    out=wAh,
                    in_=w1h[0:256, 0:H].rearrange("(i p) c -> p i c", p=128))
            xl = x1p.tile([128, 2, BLK + 1], f8, tag=f"xl{kp}", name=f"x1l_0_{kp}")
            x1dma(xl, x1l, kp, 0)
            x1kl0.append(xl)
            # per-pair halves of the quad tiles, issued alongside their
            # k-pair's x tiles so the weight stream never falls behind.
            j, s = kp // 2, 2 * (kp % 2)
            nc.sync.dma_start(
                out=w1hq[j][:, s:s + 2, :],
                in_=w1h[kp * 256:(kp + 1) * 256, :].rearrange(
                    "(i p) c -> p i c", p=128))
            nc.sync.dma_start(
                out=w1lq[j][:, s:s + 2, :],
                in_=w1l[kp * 256:(kp + 1) * 256, :].rearrange(
                    "(i p) c -> p i c", p=128))
            if kp == 1:
                nc.sync.dma_start(
                    out=b1sb, in_=b1s.rearrange("(q p) o -> p q o", p=128))
                for pq in (0, 1):
                    nc.sync.dma_start(
                        out=xt2wh[:, :, 0, pq, 0:1],
                        in_=c2h.rearrange("(a i p) o -> p a i o",
                                          p=128, a=KP2)[:, :, pq, :])
                    nc.sync.dma_start(
                        out=xt2wl[:, :, 0, pq, 0:1],
                        in_=c2l.rearrange("(a i p) o -> p a i o",
                                          p=128, a=KP2)[:, :, pq, :])

        def lhsA(kp, win, q, part, b):
            """[128, 2, 128] k-pair weight view for window win, q-tile q."""
            if kp == 0 and part == 'h' and win == 0:
                if q < 3:
                    return wfirst[:, :, q * 128:(q + 1) * 128]
                return wAh[:, :, q * 128:(q + 1) * 128]
            quad = (w1hq if part == 'h' else w1lq)[kp // 2]
            s = 2 * (kp % 2)
            c0 = win * H + q * 128
            return quad[:, s:s + 2, c0:c0 + 128]

        for b in range(NBLK):
            if b == 0:
                x1kh, x1kl = x1kh0, x1kl0
            else:
                x1kh, x1kl = [], []
                for kp in range(KP1):
                    xh = x1p.tile([128, 2, BLK + 1], f8, tag=f"xh{kp}",
                                  name=f"x1h_{b}_{kp}")
                    x1dma(xh, x1h, kp, b)
                    x1kh.append(xh)
                    xl = x1p.tile([128, 2, BLK + 1], f8, tag=f"xl{kp}",
                                  name=f"x1l_{b}_{kp}")
                    x1dma(xl, x1l, kp, b)
                    x1kl.append(xl)
            psA = [psp.tile([128, BLK], fp32, tag="mm", name=f"psA_{b}_{q}")
                   for q in range(QT1)]

            def mmA(q, kp, win, p, start, stop, b=b, x1kh=x1kh, x1kl=x1kl,
                    psA=psA):
                wpart = 'h' if p in ('hh', 'lh') else 'l'
                xt = x1kh[kp] if p in ('hh', 'hl') else x1kl[kp]
                nc.tensor.matmul(
                    psA[q], lhsT=lhsA(kp, win, q, wpart, b),
                    rhs=xt[:, :, win:win + BLK],
                    start=start, stop=stop, perf_mode=DR)

            for kp in range(KP1):
                last = (kp == KP1 - 1)
                if not last:
                    if kp == 0 and b == 0:
                        # startup wire race: run every round that needs only
                        # wfirst/wAh/x tiles before the quad-gated ones.
                        rounds = [(0, 'hh'), (0, 'lh'), (1, 'hh'), (1, 'lh'),
                                  (0, 'hl'), (1, 'hl')]
                    else:
                        rounds = [(w, p) for w in (0, 1)
                                  for p in ('hh', 'lh', 'hl')]
                    for win, p in rounds:
                        for q in range(QT1):
                            mmA(q, kp, win, p,
                                start=(kp == 0 and win == 0 and p == 'hh'),
                                stop=False)
                    continue
                # final k-pair: per-q matmuls then the bank-freeing t2 copy;
                # the hi/lo quantization chains run afterwards so all 8 PSUM
                # banks free at t2-copy rate for the next block.
                t2s = []
                for q in range(QT1):
                    for win in (0, 1):
                        for p in ('hh', 'lh', 'hl'):
                            mmA(q, kp, win, p, start=False,
                                stop=(win == 1 and p == 'hl'))
                    t2 = t2p.tile([128, BLK], fp32, tag="t2",
                                  name=f"t2_{b}_{q}")
                    nc.scalar.activation(
                        out=t2, in_=psA[q], func=Act.Identity,
                        bias=b1sb[:, q, :], scale=S2 / (SX * SW1))
                    t2s.append(t2)
                for q in range(QT1):
                    kq, pq = q // 2, q % 2
                    c0, c1 = 1 + b * BLK, 1 + (b + 1) * BLK
                    w0 = min(c1, LP2) - c0   # win0 never stores col L
                    hi0 = xt2wh[:, kq, 0, pq, c0:c0 + w0]
                    nc.scalar.activation(out=hi0, in_=t2s[q][:, 0:w0],
                                         func=Act.Identity)
                    hi1 = xt2wh[:, kq, 1, pq, c0 - 1:c1 - 1]
                    nc.scalar.activation(out=hi1, in_=t2s[q], func=Act.Identity)
                    nc.vector.tensor_sub(out=t2s[q], in0=t2s[q], in1=hi1)
                    nc.scalar.activation(
                        out=xt2wl[:, kq, 1, pq, c0 - 1:c1 - 1],
                        in_=t2s[q], func=Act.Identity)
                    nc.vector.tensor_copy(
                        out=xt2wl[:, kq, 0, pq, c0:c0 + w0],
                        in_=xt2wl[:, kq, 1, pq, c0 - 1:c0 - 1 + w0])

        # ---------------- Phase B: layer 2 + residual + RMSNorm -------------
        # token-major fp8 DoubleRow: lhsT = xt2 hi/lo k-row pairs from the
        # materialized window copies (aligned offsets), rhs = W2 hi/lo pair
        # tiles reusing the W1 quad slots. rowc/out stay bf16; the psum
        # descale folds into xres (host-scaled) and EPS_EFF.
        w2hp, w2lp = [], []
        for kp in range(KP2):
            wh = wpool.tile([128, 2, 2 * D], f8, tag=f"w{kp}", name=f"w2h_{kp}")
            nc.sync.dma_start(
                out=wh, in_=w2h[kp * 256:(kp + 1) * 256, :].rearrange(
                    "(i p) c -> p i c", p=128))
            w2hp.append(wh)
            wl = wpool.tile([128, 2, 2 * D], f8, tag=f"w{4 + kp}", name=f"w2l_{kp}")
            nc.sync.dma_start(
                out=wl, in_=w2l[kp * 256:(kp + 1) * 256, :].rearrange(
                    "(i p) c -> p i c", p=128))
            w2lp.append(wl)

        for j in range(NTT):
            tok0 = j * 128
            # the last tile drains with finer feature slices: a shorter
            # add/square chain between its final matmul and the out DMA.
            fs = FS // 2 if j == NTT - 1 else FS
            nfs = D // fs
            xr = xresp.tile([128, D], bf16, tag="xres", name=f"xres_{j}")
            nc.sync.dma_start(out=xr, in_=xres[tok0:tok0 + 128, :])
            rowc = rowp.tile([128, D], bf16, tag="rowc", name=f"rowc_{j}")
            acc = tmp.tile([128, nfs], fp32, tag="acc", name=f"acc_{j}")
            for q in range(nfs):
                sl = slice(q * fs, (q + 1) * fs)
                ps = psp.tile([128, fs], fp32, tag="mm", name=f"psB_{j}_{q}")
                first = True
                for kp in range(KP2):
                    for win in (0, 1):
                        c0 = win * D + q * fs
                        for p in ('hh', 'lh', 'hl'):
                            xt = xt2wh if p in ('hh', 'hl') else xt2wl
                            wt = (w2hp if p in ('hh', 'lh') else w2lp)[kp]
                            nc.tensor.matmul(
                                ps, lhsT=xt[:, kp, win, :, tok0:tok0 + 128],
                                rhs=wt[:, :, c0:c0 + fs],
                                start=first,
                                stop=(kp == KP2 - 1 and win == 1 and p == 'hl'),
                                perf_mode=DR)
                            first = False
                # rowc = S2*SW2*o3 slice (xres is pre-scaled on the host);
                # partial sum-of-squares right away.
                nc.vector.tensor_add(out=rowc[:, sl], in0=ps, in1=xr[:, sl])
                sq = scr.tile([128, fs], bf16, tag="sq", name=f"sq_{j}_{q}")
                nc.scalar.activation(
                    out=sq, in_=rowc[:, sl],
                    func=Act.Square, accum_out=acc[:, q:q + 1])
            # rstd_eff = 1/sqrt(S/D + EPS*(S2*SW2)^2) absorbs the descale, so
            # rowc * rstd_eff is the final normalized output.
            rstd = tmp.tile([128, 1], fp32, tag="rstd", name=f"rstd_{j}")
            nc.vector.tensor_reduce(
                out=rstd, in_=acc, axis=mybir.AxisListType.X,
                op=mybir.AluOpType.add)
            nc.scalar.activation(
                out=rstd, in_=rstd, func=Act.Sqrt, bias=epssb, scale=1.0 / D)
            nc.vector.reciprocal(out=rstd, in_=rstd)
            for q in range(nfs):
                sl = slice(q * fs, (q + 1) * fs)
                nc.vector.tensor_scalar_mul(
                    out=rowc[:, sl], in0=rowc[:, sl], scalar1=rstd)
                if (q + 1) % (nfs // 2) == 0:
                    h0 = (q + 1 - nfs // 2) * fs
                    nc.sync.dma_start(
                        out=out[tok0:tok0 + 128, h0:(q + 1) * fs],
                        in_=rowc[:, h0:(q + 1) * fs])

    nc.finalize()
    _NC_CACHE["nc"] = nc
    return nc


def _np_reference(inputs, pre_lf_indexs, out_lf_indexs, input_lf_loc, out_lf_loc,
                  inputs_loc, outputs_loc, lf1_caches, lf2_caches,
                  conv1_weight, conv2_weight, conv1_bias, conv2_bias, ln_weight):
    """Generic numpy fallback (only used if the index structure is unexpected)."""
    def fused(x, cache, pre_idx, in_lf_loc, in_loc, out_loc, W):
        bs = pre_idx.shape[0]
        xt = np.zeros((x.shape[0] + bs, x.shape[1]), x.dtype)
        xt[in_loc] = x
        xt[in_lf_loc] = cache[pre_idx]
        c = xt @ W
        h = c.shape[1] // 2
        y = c[:-1, :h] + c[1:, h:]
        return y[out_loc]

    o1 = fused(inputs, lf1_caches, pre_lf_indexs, input_lf_loc,
               inputs_loc, outputs_loc, conv1_weight) + conv1_bias
    o2 = fused(o1, lf2_caches, pre_lf_indexs, input_lf_loc,
               inputs_loc, outputs_loc, conv2_weight) + conv2_bias
    o3 = o2 + inputs
    var = np.mean(o3 * o3, axis=-1, keepdims=True)
    return (o3 / np.sqrt(var + EPS) * ln_weight).astype(np.float32)


def _split8(v, s):
    """hi/lo e4m3 pair representing v*s."""
    e4 = ml_dtypes.float8_e4m3
    hi = np.ascontiguousarray((v * s).astype(e4))
    lo = np.ascontiguousarray((v * s - hi.astype(np.float32)).astype(e4))
    return hi, lo


def kernel(**inputs):
    global LAST_EXEC_NS, LAST_RESULTS
    inp = {k: np.asarray(v) for k, v in inputs.items()}
    x = inp["inputs"].astype(np.float32, copy=False)
    lnw = inp["ln_weight"].astype(np.float32, copy=False)

    s = np.arange(BS, dtype=np.int64)
    j = np.arange(L, dtype=np.int64)
    structured = (
        np.array_equal(inp["inputs_loc"], (s[:, None] * (L + 1) + 1 + j[None, :]).reshape(-1))
        and np.array_equal(inp["outputs_loc"], (s[:, None] * (L + 1) + j[None, :]).reshape(-1))
        and np.array_equal(inp["input_lf_loc"], s * (L + 1))
    )
    if not structured:
        return _np_reference(**inp)

    from concourse.bass_utils import run_bass_kernel_spmd

    nc = _build_bass()

    bf16 = ml_dtypes.bfloat16
    pre_idx = inp["pre_lf_indexs"].astype(np.int64)
    w1hb, w1lb = _split8(inp["conv1_weight"].astype(np.float32), SW1)
    w2hb, w2lb = _split8(inp["conv2_weight"].astype(np.float32), SW2)
    b1f = np.ascontiguousarray(
        (inp["conv1_bias"].astype(np.float32) * S2).reshape(H, 1))
    b2f = inp["conv2_bias"].astype(np.float32)

    in_maps = []
    for sq in range(BS):
        xs = x[sq * L:(sq + 1) * L]                       # [2048, 2048]
        a = np.empty((D, L + 1), np.float32)
        a[:, 0] = inp["lf1_caches"][pre_idx[sq]]
        a[:, 1:] = xs.T
        xh, xl = _split8(a, SX)
        ch, cl = _split8(
            inp["lf2_caches"][pre_idx[sq]].astype(np.float32).reshape(H, 1), S2)
        in_maps.append({
            "x1h": xh, "x1l": xl,
            "xres": np.ascontiguousarray(
                ((xs + b2f[None, :]) * (S2 * SW2)).astype(bf16)),
            "c2h": ch, "c2l": cl,
            "w1h": w1hb, "w1l": w1lb,
            "w2h": w2hb, "w2l": w2lb,
            "b1s": b1f,
        })

    res = run_bass_kernel_spmd(nc, in_maps, list(range(NCORES)), trace=TRACE)
    LAST_EXEC_NS = res.exec_time_ns
    LAST_RESULTS = res
    out = np.concatenate(
        [res.results[i]["out"].astype(np.float32) for i in range(NCORES)], axis=0)
    if not np.all(lnw == 1.0):
        out = out * lnw[None, :]
    return out.astype(np.float32)
